# revision 38
# baseline (speedup 1.0000x reference)
"""Low-rank ray tracer CSI kernel for 8 Trainium2 NeuronCores.

v13: fp8 error-feedback stream + DoubleRow p-fold + split final chain.

    csi[k] = (1/D) * f_k^T (Ua^T Ub) f_k,   Ua[d,r] = sum_p ua[d,p,r]

The kernel is HBM-stream-bound (~170 GB/s ramping to ~420 GB/s/core; the
chip power-throttles to a ~50% utilization cap for ~27% of the run with all
8 cores streaming), so the main lever is bytes: ua/ub ship as fp8 e4m3
(4.2 MB/core vs 8.4 fp16).  Plain e4m3 rounding fails the 2e-2 gate
(2.9e-2 end-to-end); the host instead quantizes with error feedback along p
(q[p] = e4m3(x[p] + carry)), which telescopes the p-sum error to the final
carry — 1.8e-3 end-to-end on the harness inputs.

PE can't matmul int8, but fp8e4 DoubleRow contracts 2 k-tiles per pass:
with a doubled identity [128, 2, 128] stationary, out = X[:,0,:] + X[:,1,:]
— a pairwise p-add at 2x fp16 throughput (HW-validated exact).  Host layout
[D, R, 4, 2, 32] (p = j*64 + t*32 + f) keeps every r-chunk slice contiguous
per partition (128 descriptors per DMA kick, ~650ns — 512B-run slices cost
1-4us per kick and starve the stream).  DVE direct-reduces the j=3 slab of
the big ua chunks (merged as a second accumulating Gram matmul) so the
throttled PE keeps pace with the stream.

The stream sends ALL of ua, then ub with a small last chunk: the Gram /
g-matmul / mul / reduce chain splits into an r2<56 part that overlaps the
last ub folds (its csi half DMAs out immediately; the host adds the two
halves) and an 8-wide part2, so only ~1.5us of final chain trails the last
byte.  A burst of dummy DR matmuls bridges PE from kernel start to the
first chunk's arrival so the fold pipeline starts warm.
"""

import sys

import ml_dtypes
import numpy as np

sys.path.insert(0, "/opt/trn_rl_repo")

import concourse.bacc as bacc
import concourse.bass as bass
import concourse.mybir as mybir
from concourse.bass_utils import run_bass_kernel_spmd

from concourse.tile import TileContext

D, P, R, K = 1024, 256, 64, 1024
NCORES = 8
DC = D // NCORES  # directions per core
RCS_A = (8, 24, 24, 8)  # ua r-chunks (small first: overlap the DMA ramp)
RCS_B = (24, 24, 8, 8)  # ub r-chunks (small last: short drain + split final)
R1 = 48  # final-chain split point: part1 (r2<48) runs while ub chunks 3-4 stream
KC = K // 128  # k chunks of 128 (PSUM partition limit)
PF = 32  # p-fold tail width
NWARM = 12  # dummy DR matmuls: ramp the PE pstate AND bridge the idle gap
# until the first chunk lands (~11us) so the fold pipeline never droops

F32 = mybir.dt.float32
F16 = mybir.dt.float16
F8 = mybir.dt.float8e4
E4M3 = ml_dtypes.float8_e4m3


def build_bass() -> bass.Bass:
    nc = bacc.Bacc(None, target_bir_lowering=False)
    # per-core shards, host layout [d, r, j, t, f] with p = j*64 + t*32 + f,
    # fp8 e4m3 (error-feedback quantized along p on host)
    ua = nc.declare_dram_parameter("ua", [DC, R, 4, 2, PF], F8, isOutput=False)
    ub = nc.declare_dram_parameter("ub", [DC, R, 4, 2, PF], F8, isOutput=False)
    ft = nc.declare_dram_parameter("ft", [R, K], F16, isOutput=False)
    # F as [p, c, r] blocks (k = c*128 + p), prebuilt on host
    fpc = nc.declare_dram_parameter("fpc", [128, KC, R], F16, isOutput=False)
    # out[p, c] = partial csi[c*128 + p] from r2 < R1, already scaled by 1/D;
    # out2 = the r2 >= R1 remainder (host adds them — it sums cores anyway).
    # Shipping the parts separately lets part1's DMA overlap part2's compute.
    out = nc.declare_dram_parameter("out", [128, KC], F32, isOutput=True)
    out2 = nc.declare_dram_parameter("out2", [128, KC], F32, isOutput=True)

    with TileContext(nc) as tc:
        with (
            tc.tile_pool(name="const", bufs=1) as cpool,
            tc.tile_pool(name="chunks", bufs=8) as chpool,
            tc.tile_pool(name="small", bufs=1) as spool,
            tc.tile_pool(name="scratch", bufs=1) as scpool,
            tc.tile_pool(name="pwarm", bufs=1, space="PSUM") as wpool,
            tc.tile_pool(name="pfold", bufs=4, space="PSUM") as fpool,
            tc.tile_pool(name="pfinal", bufs=1, space="PSUM") as ppool1,
        ):
            # doubled identity [128, 2, 128] fp8: dident[i, t, m] = (i == m);
            # DoubleRow with it as stationary computes X[:,0,:] + X[:,1,:]
            dident = cpool.tile([128, 2, 128], F8)
            nc.gpsimd.memset(dident[:], 0.0)
            nc.gpsimd.affine_select(
                out=dident[:],
                in_=dident[:],
                compare_op=mybir.AluOpType.not_equal,
                fill=1.0,
                base=0,
                pattern=[[0, 2], [-1, 128]],
                channel_multiplier=1,
            )

            # PE pstate warmup while the first chunk is in flight
            warm = wpool.tile([128, 128], F32)
            for _ in range(NWARM):
                nc.tensor.matmul(
                    warm[:],
                    dident[:],
                    dident[:],
                    start=True,
                    stop=True,
                    perf_mode=mybir.MatmulPerfMode.DoubleRow,
                )

            ft_sb = cpool.tile([R, K], F16)
            fpc_sb = cpool.tile([128, KC, R], F16)
            nc.scalar.dma_start(out=ft_sb[:], in_=ft[:])
            nc.scalar.dma_start(out=fpc_sb[:], in_=fpc[:])

            u_a = spool.tile([DC, R], F16, tag="u_a")
            u_b = spool.tile([DC, R], F16, tag="u_b")
            # DVE-side partial p-sums (j=3 slabs of the big ua chunks);
            # merged via a second accumulating gram matmul
            u_s = spool.tile([DC, R], F16, tag="u_s")
            nc.gpsimd.memset(u_s[:], 0.0)
            m_psum = ppool1.tile([R, R], F32, tag="m")
            m_sb = spool.tile([R, R], F16)
            g_all = ppool1.tile([128, KC, R], F32, tag="g_all")
            scr = scpool.tile([128, KC, R], F16, tag="scr")
            csi = spool.tile([128, KC], F32, tag="csi")
            csi2 = spool.tile([128, KC], F32, tag="csi2")

            def fold_chunk(t_ap, u, r0, rc, assist=False):
                """DMA one [DC, rc] r-chunk and p-fold it: 256 -> 32 on PE
                (DoubleRow), 32 -> 1 on DVE.  With assist, DVE takes the j=3
                slab directly (into u_s) to keep PE ahead of the stream."""
                ch = chpool.tile([DC, rc, 4, 2, PF], F8, tag="chunk")
                nc.sync.dma_start(out=ch[:], in_=t_ap[:, r0 : r0 + rc])
                nj = 3 if assist else 4
                if assist:
                    nc.vector.tensor_reduce(
                        out=u_s[:, r0 : r0 + rc],
                        in_=ch[:, :, 3],
                        axis=mybir.AxisListType.XY,
                        op=mybir.AluOpType.add,
                    )
                for g0 in range(0, rc, 16):
                    rg = min(16, rc - g0)
                    fold = fpool.tile([DC, rg, PF], F32, tag="fold")
                    for j in range(nj):
                        nc.tensor.matmul(
                            fold[:].rearrange("q r f -> q (r f)"),
                            dident[:],
                            ch[:, g0 : g0 + rg, j].rearrange("q r t f -> q t r f"),
                            start=(j == 0),
                            stop=(j == nj - 1),
                            perf_mode=mybir.MatmulPerfMode.DoubleRow,
                        )
                    nc.vector.tensor_reduce(
                        out=u[:, r0 + g0 : r0 + g0 + rg],
                        in_=fold[:],
                        axis=mybir.AxisListType.X,
                        op=mybir.AluOpType.add,
                    )

            def final_part(r0, r1):
                """Gram columns [r0:r1), their g-matmuls, and the partial csi
                contribution (into csi for part 1, csi2 for part 2)."""
                nc.tensor.matmul(
                    m_psum[:, r0:r1], u_a[:], u_b[:, r0:r1], start=True, stop=False
                )
                nc.tensor.matmul(
                    m_psum[:, r0:r1], u_s[:], u_b[:, r0:r1], start=False, stop=True
                )
                nc.vector.tensor_scalar_mul(m_sb[:, r0:r1], m_psum[:, r0:r1], 1.0 / D)
                for c in range(KC):
                    nc.tensor.matmul(
                        g_all[:, c, r0:r1],
                        ft_sb[:, c * 128 : (c + 1) * 128],
                        m_sb[:, r0:r1],
                        start=True,
                        stop=True,
                    )
                nc.vector.tensor_mul(
                    out=scr[:, :, r0:r1], in0=g_all[:, :, r0:r1], in1=fpc_sb[:, :, r0:r1]
                )
                nc.vector.tensor_reduce(
                    out=(csi[:] if r0 == 0 else csi2[:]),
                    in_=scr[:, :, r0:r1],
                    axis=mybir.AxisListType.X,
                    op=mybir.AluOpType.add,
                )

            with nc.allow_low_precision(reason="fp8 EF p-sums; fp16 finals; gate 2e-2"):
                r0 = 0
                for rc in RCS_A:
                    fold_chunk(ua, u_a, r0, rc, assist=(rc > 8))
                    r0 += rc
                r0 = 0
                for ci, rc in enumerate(RCS_B):
                    fold_chunk(ub, u_b, r0, rc)
                    r0 += rc
                    if r0 == R1:
                        # r2 < R1 of the final chain runs under the last ub
                        # chunk's DMA + fold
                        final_part(0, R1)
                        nc.scalar.dma_start(out=out[:], in_=csi[:])
                final_part(R1, R)
            nc.scalar.dma_start(out=out2[:], in_=csi2[:])
    nc.compile()
    return nc


_NC_CACHE = None


EXTRA_INPUT_NAMES = ("ft", "fpc")


def _ef_quant_t(x: np.ndarray) -> np.ndarray:
    """Error-feedback e4m3 quantization along p of [D, P, R] input; returns
    a [D, R, 4, 2, 32] fp8 array (p = j*64 + t*32 + f).  sum_p q[d,p,r] =
    sum_p x[d,p,r] - final_carry[d,r], so the p-sum error is one rounding
    step, not sqrt(P) accumulated noise."""
    q = np.empty((x.shape[0], x.shape[2], x.shape[1]), dtype=E4M3)  # [D, R, P]
    carry = np.zeros((x.shape[0], x.shape[2]), dtype=np.float32)
    for p in range(x.shape[1]):
        v = x[:, p, :] + carry
        qv = v.astype(E4M3)
        carry = v - qv.astype(np.float32)
        q[:, :, p] = qv
    # [D, R, P] -> [D, R, 4, 2, 32]
    q = q.reshape(x.shape[0], x.shape[2], 4, 2, PF)
    return np.ascontiguousarray(q)


def _prep(inputs):
    ua = np.asarray(inputs["attenuation_vectors"], dtype=np.float32)
    ub = np.asarray(inputs["radiation_vectors"], dtype=np.float32)
    f = np.ascontiguousarray(inputs["frequency_basis_vectors"], dtype=np.float32)

    ua_t = _ef_quant_t(ua)
    ub_t = _ef_quant_t(ub)
    fh = f.astype(np.float16)
    ft = np.ascontiguousarray(fh.T)
    fpc = np.ascontiguousarray(fh.reshape(KC, 128, R).transpose(1, 0, 2))
    return ua_t, ub_t, ft, fpc


def kernel(**inputs: np.ndarray) -> np.ndarray:
    global _NC_CACHE
    ua_t, ub_t, ft, fpc = _prep(inputs)

    if _NC_CACHE is None:
        _NC_CACHE = build_bass()
    nc = _NC_CACHE

    in_maps = [
        {
            "ua": ua_t[c * DC : (c + 1) * DC],
            "ub": ub_t[c * DC : (c + 1) * DC],
            "ft": ft,
            "fpc": fpc,
        }
        for c in range(NCORES)
    ]
    res = run_bass_kernel_spmd(nc, in_maps, list(range(NCORES)))
    acc = np.zeros((128, KC), dtype=np.float32)
    for r in res.results:
        acc += r["out"]
        acc += r["out2"]
    return acc.T.reshape(K).astype(np.float32)


if __name__ == "__main__":
    rng = np.random.default_rng(0)
    ins = {
        "attenuation_vectors": rng.standard_normal((D, P, R), dtype=np.float32),
        "radiation_vectors": rng.standard_normal((D, P, R), dtype=np.float32),
        "frequency_basis_vectors": rng.standard_normal((K, R), dtype=np.float32),
    }
    got = kernel(**ins)
    ua_s = ins["attenuation_vectors"].sum(axis=1)
    ub_s = ins["radiation_vectors"].sum(axis=1)
    a = ua_s @ ins["frequency_basis_vectors"].T
    b = ub_s @ ins["frequency_basis_vectors"].T
    want = (a * b).sum(axis=0) / D
    err = np.abs(got - want).max() / np.abs(want).max()
    print("rel err vs local numpy:", err)
